# revision 4
# baseline (speedup 1.0000x reference)
"""Trainium2 Bass kernel for nn_MelPCENPreprocessor.

Pipeline: audio (N,32000) -> reflect-pad -> STFT(400/160, hann) power
-> mel(128) -> PCEN (IIR smooth + pointwise) -> bilinear resize (201->192)
-> (N,1,192,128).

Mapping:
  * Host prep restructures the hop-160 framing into 3 strided layouts
    (t0/tb/aux) so the windowed DFT becomes 4 full-K matmul chunks
    (K=128,128,112,32) accumulated in PSUM. Frequency bins 0 and 200
    carry zero mel weight and are dropped (398 = cos|sin of f=1..199
    packed as [cos 1..128 | sin 1..128 | cos 129..199 | sin 129..199]).
  * Two samples per "pair" -> moving dim 404 (junk cols at 201/202/403).
  * DFT runs as bf16 hi/lo 3-term compensation (Wh@xh + Wh@xl + Wl@xh,
    exact bf16 products accumulated in fp32 PSUM; dropped Wl@xl term is
    ~2^-16 relative). Host supplies pre-split bf16 hi/lo input layouts
    (same total bytes as f32).
  * power = cos^2 + sin^2 via ACT Square (PSUM->SBUF) + DVE add.
  * mel = FB^T @ power in float32r (FB sparsity keeps the f32r rounding
    ~5e-5 of scale).
  * PCEN IIR M'[t] = (1-s)M'[t-1] + E[t] via DVE tensor_tensor_scan
    (M = s*M' folded into the Ln activation scale).
  * pcen = sqrt(mel*exp(-0.8*ln(s*M'+eps)) + 2) via ACT Ln/Exp/Sqrt + DVE mul.
  * PE transpose [mel,t] -> [t,mel], then resize as plain fp32 matmul
    against the jax-style antialiased bilinear matrix; the final -sqrt(2)
    is a DVE tensor_scalar_add during PSUM->SBUF eviction.

Per core: N/8 samples, pure data parallel, no collectives.
"""
import numpy as np
import ml_dtypes

import concourse.bass as bass
import concourse.bacc as bacc
import concourse.mybir as mybir
from concourse import tile
from concourse.bass_utils import run_bass_kernel_spmd

SR = 16000
N_FFT = 400
HOP = 160
N_MELS = 128
F_MAX = 8000.0
S = 0.04
ALPHA = 0.8
DELTA = 2.0
FLOOR = 1e-08
T = 201           # frames per sample
TT = 192          # resized time
PAD = 200
COLS = 203        # staged columns per sample
NW = 404          # moving dim per 2-sample pair
F32 = mybir.dt.float32
F32R = mybir.dt.float32r
BF16 = mybir.dt.bfloat16
BF16NP = ml_dtypes.bfloat16

# DFT K-chunks: (layout, rows, col_shift). k-coverage:
#   t0  d0 -> k in [0,128)          t0  d1 -> k in [160,288)
#   aux d0 -> k in [320,400)+[128,160)   tb d1 -> k in [288,320)
CHUNKS = [("t0", 128, 0), ("t0", 128, 1), ("aux", 112, 0), ("tb", 32, 1)]
MC = [(0, 128), (128, 128), (256, 71), (327, 71)]  # freq col chunks of W
SQRT2 = float(np.sqrt(2.0))


# ---------------- constant matrices (host, fp64 -> fp32) ----------------

def _hann():
    n = np.arange(N_FFT)
    return 0.5 * (1.0 - np.cos(2.0 * np.pi * n / N_FFT))


def _mel_fb():
    n_freqs = N_FFT // 2 + 1
    all_freqs = np.linspace(0.0, SR / 2, n_freqs)

    def h2m(f):
        return 2595.0 * np.log10(1.0 + f / 700.0)

    m_pts = np.linspace(h2m(0.0), h2m(F_MAX), N_MELS + 2)
    f_pts = 700.0 * (10.0 ** (m_pts / 2595.0) - 1.0)
    f_diff = f_pts[1:] - f_pts[:-1]
    slopes = f_pts[None, :] - all_freqs[:, None]
    down = -slopes[:, :-2] / f_diff[:-1]
    up = slopes[:, 2:] / f_diff[1:]
    return np.maximum(0.0, np.minimum(down, up)).astype(np.float32)  # (201,128)


def _dft_w():
    k = np.arange(N_FFT)[:, None]
    h = _hann()[:, None]
    f_lo = np.arange(1, 129)[None, :]
    f_hi = np.arange(129, 200)[None, :]
    a_lo = 2.0 * np.pi * k * f_lo / N_FFT
    a_hi = 2.0 * np.pi * k * f_hi / N_FFT
    return np.concatenate(
        [h * np.cos(a_lo), h * np.sin(a_lo),
         h * np.cos(a_hi), h * np.sin(a_hi)], axis=1).astype(np.float32)  # (400,398)


def _resize_r():
    scale = TT / T
    sample_f = (np.arange(TT, dtype=np.float64) + 0.5) / scale - 0.5
    j = np.arange(T, dtype=np.float64)[None, :]
    w = np.maximum(0.0, 1.0 - np.abs((j - sample_f[:, None]) * scale))
    w = w / w.sum(axis=1, keepdims=True)
    return w.astype(np.float32)  # (192, 201), rows sum to 1


def _k_rows(name, delta):
    if name == "t0":
        base = np.arange(128)
    elif name == "tb":
        base = 128 + np.arange(32)
    else:
        base = np.concatenate([320 + np.arange(80), 128 + np.arange(32)])
    return base + HOP * delta


def _hi_lo(x):
    hi = x.astype(BF16NP)
    lo = (x - hi.astype(np.float32)).astype(BF16NP)
    return hi, lo


def _consts():
    W = _dft_w()
    fb = _mel_fb()
    R = _resize_r()
    def zpad(a, rows):
        out = np.zeros((rows, a.shape[1]), a.dtype)
        out[:a.shape[0]] = a
        return out

    c = {}
    for i, (name, rows, delta) in enumerate(CHUNKS):
        wh, wl = _hi_lo(np.ascontiguousarray(W[_k_rows(name, delta)]))
        if name == "tb":
            # packed 3-term chunk: rhs rows [xh; xl; xh] pair with [Wh; Wh; Wl];
            # zero-padded to K=128 so FWL (4 rows/cycle weight load) triggers
            c["wtbs"] = zpad(np.concatenate([wh, wh, wl], axis=0), 128)
        elif name == "aux":
            c[f"w{i}h"] = zpad(wh, 128)
            c[f"w{i}l"] = zpad(wl, 128)
        else:
            c[f"w{i}h"] = wh
            c[f"w{i}l"] = wl
    c["fb0"] = np.ascontiguousarray(fb[1:129])          # (128,128)
    c["fb1"] = np.ascontiguousarray(fb[129:200])        # (71,128)
    RT = np.ascontiguousarray(R.T)                      # (201,192)
    h0, l0 = _hi_lo(np.ascontiguousarray(RT[0:128]))
    h1, l1 = _hi_lo(np.ascontiguousarray(RT[128:201]))
    c["rt0h"], c["rt0l"], c["rt1h"], c["rt1l"] = h0, l0, h1, l1
    c["ident"] = np.eye(128, dtype=np.float32).astype(BF16NP)
    return c


CONST_DTYPES = {"w0h": BF16, "w0l": BF16, "w1h": BF16, "w1l": BF16,
                "w2h": BF16, "w2l": BF16, "wtbs": BF16,
                "fb0": F32R, "fb1": F32R,
                "rt0h": BF16, "rt0l": BF16, "rt1h": BF16, "rt1l": BF16,
                "ident": BF16}
CONST_SHAPES = {"w0h": (128, 398), "w0l": (128, 398), "w1h": (128, 398),
                "w1l": (128, 398), "w2h": (128, 398), "w2l": (128, 398),
                "wtbs": (128, 398), "fb0": (128, 128),
                "fb1": (71, 128), "rt0h": (128, 192), "rt0l": (128, 192),
                "rt1h": (73, 192), "rt1l": (73, 192), "ident": (128, 128)}


# ---------------- host input staging ----------------

def _stage(audio):
    """audio (N,32000) f32 -> dict of 6 bf16 hi/lo layouts."""
    N = audio.shape[0]
    xp = np.pad(audio, ((0, 0), (PAD, PAD)), mode="reflect")
    st = xp.strides

    def lay(base, rows):
        v = np.lib.stride_tricks.as_strided(
            xp[:, base:], shape=(N, rows, COLS), strides=(st[0], st[1], st[1] * HOP))
        return np.ascontiguousarray(v)

    def zpad3(a, rows):
        out = np.zeros((a.shape[0], rows, a.shape[2]), a.dtype)
        out[:, :a.shape[1]] = a
        return out

    out = {}
    for name, arr in (("t0", lay(0, 128)),
                      ("aux", np.concatenate([lay(320, 80), lay(128, 32)], axis=1))):
        hi, lo = _hi_lo(arr)
        if name == "aux":
            hi, lo = zpad3(hi, 128), zpad3(lo, 128)
        out[f"{name}h"] = hi
        out[f"{name}l"] = lo
    hi, lo = _hi_lo(lay(128, 32))
    out["tbs"] = zpad3(np.concatenate([hi, lo, hi], axis=1), 128)  # (N,128,203)
    return out


# ---------------- device program ----------------

def emit_block(nc, tc, csb, c96, floor_c, delta_c, din, dout, pools,
               lay_rows, pairs):
    """Per-pair pipeline. All ACT functions (Square/Ln/Exp) live in one
    activation-table set (see _dedupe_act_loads), so no sweep batching is
    needed."""
    (xpool, wpool, opool, ps_dft, ps_mel, ps_tr, ps_rz) = pools
    A = mybir.ActivationFunctionType

    terms = []
    for ci, (name, rows, delta) in enumerate(CHUNKS):
        if name == "tb":
            terms.append(("wtbs", "tbs", 128, delta))
        else:
            terms.append((f"w{ci}h", f"{name}h", 128, delta))
            terms.append((f"w{ci}h", f"{name}l", 128, delta))
            terms.append((f"w{ci}l", f"{name}h", 128, delta))

    for p in pairs:
        n0 = 2 * p
        # ---- loads ----
        xt = {}
        for name in din:
            r = lay_rows[name]
            xtile = xpool.tile([r, 2 * COLS], BF16, tag=f"x_{name}",
                               name=f"x_{name}")
            nc.sync.dma_start(
                xtile[:, :].rearrange("p (s u) -> p s u", s=2),
                din[name][n0:n0 + 2].rearrange("s p u -> p s u"))
            xt[name] = xtile

        # ---- DFT (bf16 hi/lo 3-term; tb chunk K-packed) ----
        dft = [ps_dft.tile([mw, NW], F32, tag=f"dft{mi}", name=f"dft{mi}")
               for mi, (mo, mw) in enumerate(MC)]
        for mi, (mo, mw) in enumerate(MC):
            for ti, (wsb, xsb, rows, delta) in enumerate(terms):
                nc.tensor.matmul(
                    dft[mi][:, :],
                    csb[wsb][0:rows, mo:mo + mw],
                    xt[xsb][0:rows, delta:delta + NW],
                    start=(ti == 0), stop=(ti == len(terms) - 1))

        # ---- power ----
        pw0 = wpool.tile([128, NW], F32R, tag="pw0", name="pw0")
        pw1 = wpool.tile([71, NW], F32R, tag="pw1", name="pw1")
        sq0 = wpool.tile([128, NW], F32R, tag="sq0", name="sq0")
        sq1 = wpool.tile([71, NW], F32R, tag="sq1", name="sq1")
        nc.scalar.activation(pw0[:, :], dft[0][:, :], A.Square)
        nc.scalar.activation(sq0[:, :], dft[1][:, :], A.Square)
        nc.scalar.activation(pw1[:, :], dft[2][:, :], A.Square)
        nc.scalar.activation(sq1[:, :], dft[3][:, :], A.Square)
        nc.vector.tensor_add(pw0[:, :], pw0[:, :], sq0[:, :])
        nc.vector.tensor_add(pw1[:, :], pw1[:, :], sq1[:, :])

        # ---- mel (f32r) ----
        mel = ps_mel.tile([128, NW], F32, tag="mel", name="mel")
        nc.tensor.matmul(mel[:, :], csb["fb0"][:, :], pw0[:, :],
                         start=True, stop=False)
        nc.tensor.matmul(mel[:, :], csb["fb1"][:, :], pw1[:, :],
                         start=False, stop=True)

        # ---- PCEN scan ----
        init = wpool.tile([128, 2], F32, tag="init", name="init", bufs=2)
        nc.vector.tensor_scalar_mul(init[:, 0:1], mel[:, 0:1], 1.0 / S)
        nc.vector.tensor_scalar_mul(init[:, 1:2], mel[:, COLS:COLS + 1], 1.0 / S)
        mp = wpool.tile([128, NW], F32, tag="mp", name="mp")
        nc.vector.tensor_tensor_scan(
            mp[:, 0:COLS], c96[:, 0:COLS], mel[:, 0:COLS], init[:, 0:1],
            mybir.AluOpType.mult, mybir.AluOpType.add)
        nc.vector.tensor_tensor_scan(
            mp[:, COLS:NW], c96[:, 0:T], mel[:, COLS:NW], init[:, 1:2],
            mybir.AluOpType.mult, mybir.AluOpType.add)
        melc = wpool.tile([128, NW], F32, tag="melc", name="melc")
        nc.vector.tensor_copy(melc[:, :], mel[:, :])

        # ---- PCEN pointwise; sqrt(y) = exp(0.5*ln(y)) stays in-table ----
        t1 = wpool.tile([128, NW], F32, tag="t1", name="t1")
        t2 = wpool.tile([128, NW], F32, tag="t2", name="t2")
        t4 = wpool.tile([128, NW], F32, tag="t4", name="t4")
        nc.scalar.activation(t1[:, :], mp[:, :], A.Ln,
                             bias=floor_c[:, 0:1], scale=S)
        nc.scalar.activation(t2[:, :], t1[:, :], A.Exp, scale=-ALPHA)
        nc.vector.tensor_mul(t2[:, :], melc[:, :], t2[:, :])
        nc.scalar.activation(t1[:, :], t2[:, :], A.Ln, bias=delta_c[:, 0:1])
        nc.scalar.activation(t4[:, :], t1[:, :], A.Exp, scale=0.5)
        t4h = wpool.tile([128, NW], BF16, tag="t4h", name="t4h")
        t4l = wpool.tile([128, NW], BF16, tag="t4l", name="t4l")
        nc.vector.tensor_copy(t4h[:, :], t4[:, :])
        nc.vector.tensor_sub(t4l[:, :], t4[:, :], t4h[:, :])

        # ---- dual bf16 transposes ----
        trh = ps_tr.tile([128, 512], BF16, tag="trh", name="trh")
        trl = ps_tr.tile([128, 512], BF16, tag="trl", name="trl")
        for tr, t4x in ((trh, t4h), (trl, t4l)):
            nc.tensor.transpose(tr[0:128, 0:128], t4x[:, 0:128],
                                csb["ident"][:, :])
            nc.tensor.transpose(tr[0:73, 128:256], t4x[:, 128:201],
                                csb["ident"][:, :])
            nc.tensor.transpose(tr[0:128, 256:384], t4x[:, COLS:COLS + 128],
                                csb["ident"][:, :])
            nc.tensor.transpose(tr[0:73, 384:512],
                                t4x[:, COLS + 128:COLS + 201],
                                csb["ident"][:, :])
        p1h = wpool.tile([128, 256], BF16, tag="p1h", name="p1h", bufs=3)
        p1l = wpool.tile([128, 256], BF16, tag="p1l", name="p1l", bufs=3)
        p2h = wpool.tile([73, 256], BF16, tag="p2h", name="p2h", bufs=3)
        p2l = wpool.tile([73, 256], BF16, tag="p2l", name="p2l", bufs=3)
        for pt, tr in ((p1h, trh), (p1l, trl)):
            nc.vector.tensor_copy(pt[:, 0:128], tr[0:128, 0:128])
            nc.vector.tensor_copy(pt[:, 128:256], tr[0:128, 256:384])
        for pt, tr in ((p2h, trh), (p2l, trl)):
            nc.vector.tensor_copy(pt[:, 0:128], tr[0:73, 128:256])
            nc.vector.tensor_copy(pt[:, 128:256], tr[0:73, 384:512])

        # ---- resize (bf16 3-term) + store ----
        rz = ps_rz.tile([128, 512], F32, tag="rz", name="rz")
        rmm = [("rt0h", p1h), ("rt0l", p1h), ("rt0h", p1l),
               ("rt1h", p2h), ("rt1l", p2h), ("rt1h", p2l)]
        for gi, msl in enumerate((slice(0, 128), slice(128, 192))):
            rows = 128 if gi == 0 else 64
            ps_out = rz[0:rows, 256 * gi:256 * gi + 256]
            for ti, (wn, pt) in enumerate(rmm):
                nc.tensor.matmul(ps_out, csb[wn][:, msl], pt[:, :],
                                 start=(ti == 0), stop=(ti == len(rmm) - 1))
        o1 = opool.tile([128, 256], F32, tag="o1", name="o1")
        o2 = opool.tile([64, 256], F32, tag="o2", name="o2")
        nc.vector.tensor_scalar_add(o1[:, :], rz[0:128, 0:256], -SQRT2)
        nc.vector.tensor_scalar_add(o2[:, :], rz[0:64, 256:512], -SQRT2)
        nc.sync.dma_start(
            dout[n0:n0 + 2, 0:128, :].rearrange("s t m -> t s m"),
            o1[:, :].rearrange("t (s m) -> t s m", s=2))
        nc.sync.dma_start(
            dout[n0:n0 + 2, 128:TT, :].rearrange("s t m -> t s m"),
            o2[:, :].rearrange("t (s m) -> t s m", s=2))


BLOCK = 16


def _build_program(nper):
    """Build the per-core program for nper samples (nper even)."""
    assert nper % 2 == 0
    npair = nper // 2
    nc = bacc.Bacc("TRN2", target_bir_lowering=False, debug=False,
                   num_devices=1)

    lay_rows = {"t0h": 128, "t0l": 128, "auxh": 128, "auxl": 128, "tbs": 128}
    din = {name: nc.dram_tensor(name, [nper, r, COLS], BF16,
                                kind="ExternalInput")
           for name, r in lay_rows.items()}
    dc = {k: nc.dram_tensor(k, list(CONST_SHAPES[k]), CONST_DTYPES[k],
                            kind="ExternalInput")
          for k in CONST_SHAPES}
    dout = nc.dram_tensor("out", [nper, TT, 128], F32, kind="ExternalOutput")

    with tile.TileContext(nc) as tc:
        with (
            tc.tile_pool(name="const", bufs=1) as cpool,
            tc.tile_pool(name="xin", bufs=6) as xpool,
            tc.tile_pool(name="work", bufs=8) as wpool,
            tc.tile_pool(name="outs", bufs=3) as opool,
            tc.tile_pool(name="ps_dft", bufs=1, space="PSUM") as ps_dft,
            tc.tile_pool(name="ps_mel", bufs=1, space="PSUM") as ps_mel,
            tc.tile_pool(name="ps_tr", bufs=1, space="PSUM") as ps_tr,
            tc.tile_pool(name="ps_rz", bufs=1, space="PSUM") as ps_rz,
        ):
            # constants into SBUF
            csb = {}
            for k, shp in CONST_SHAPES.items():
                cdt = CONST_DTYPES[k]
                t = cpool.tile(list(shp), cdt, tag=k, name=f"c_{k}")
                nc.sync.dma_start(t[:, :], dc[k][:, :])
                csb[k] = t
            c96 = cpool.tile([128, NW], F32, tag="c96")
            nc.vector.memset(c96[:, :], 1.0 - S)
            floor_c = cpool.tile([128, 1], F32, tag="floor_c")
            nc.vector.memset(floor_c[:, :], FLOOR)
            delta_c = cpool.tile([128, 1], F32, tag="delta_c")
            nc.vector.memset(delta_c[:, :], DELTA)

            pools = (xpool, wpool, opool, ps_dft, ps_mel, ps_tr, ps_rz)
            for b0 in range(0, npair, BLOCK):
                pairs = list(range(b0, min(b0 + BLOCK, npair)))
                emit_block(nc, tc, csb, c96, floor_c, delta_c, din, dout,
                           pools, lay_rows, pairs)

    nc.finalize()
    _dedupe_act_loads(nc)
    return nc


def _dedupe_act_loads(nc):
    """All activations used here (Square/Ln/Exp) live in one table set
    (natural_log_exp_and_others); point the first LoadActFuncSet of each
    block at it and drop the redundant reloads the generic chooser
    emitted (1.28us each on ACT)."""
    from concourse.hw_specs import get_activation_tables
    import concourse.mybir as _mb
    A = _mb.ActivationFunctionType
    tables = get_activation_tables(nc.m.arch)
    set_id = None
    for i, (name, s) in enumerate(tables.items()):
        if {A.Square, A.Ln, A.Exp} <= s:
            set_id = i
            break
    assert set_id is not None
    for blk in nc.m.functions[0].blocks:
        keep = []
        seen = False
        for inst in blk.instructions:
            if type(inst).__name__ == "InstLoadActFuncSet":
                si = inst.sync_info
                if si is not None and (si.on_wait or si.on_update):
                    inst.act_func_set_id = set_id
                    keep.append(inst)
                    seen = True
                elif not seen:
                    inst.act_func_set_id = set_id
                    keep.append(inst)
                    seen = True
                # else: drop redundant load
            else:
                keep.append(inst)
        blk.instructions[:] = keep
    return nc


_CACHE = {}
LAST_EXEC_NS = None


def _install_ntff_shim():
    """Profiling-only (KERNEL_TRACE=1): provide antenv.axon_hooks so
    bass_utils' trace path can reach the axon NTFF profiler."""
    import sys
    import types
    if "antenv.axon_hooks" in sys.modules:
        return
    try:
        from trn_agent_boot.trn_boot import _ntff_profile_via_ctypes
        hook = _ntff_profile_via_ctypes("/opt/axon/libaxon_pjrt.so")
    except Exception:
        hook = None
    mod = types.ModuleType("antenv.axon_hooks")
    mod.get_axon_ntff_profile_hook = lambda: hook
    sys.modules["antenv.axon_hooks"] = mod


def _program(nper):
    if nper not in _CACHE:
        _CACHE[nper] = _build_program(nper)
    return _CACHE[nper]


def kernel(audio):
    global LAST_EXEC_NS
    import os
    audio = np.ascontiguousarray(np.asarray(audio, dtype=np.float32))
    N = audio.shape[0]
    n_cores = 8 if N % 16 == 0 else 1
    nper = N // n_cores
    staged = _stage(audio)
    consts = _consts()
    nc = _program(nper)
    in_maps = []
    for c in range(n_cores):
        sl = slice(c * nper, (c + 1) * nper)
        m = {k: v[sl] for k, v in staged.items()}
        m.update(consts)
        in_maps.append(m)
    trace = bool(os.environ.get("KERNEL_TRACE"))
    if trace:
        _install_ntff_shim()
    r = run_bass_kernel_spmd(nc, in_maps, list(range(n_cores)), trace=trace)
    if trace:
        LAST_EXEC_NS = r.exec_time_ns
        if r.instructions_and_trace is not None:
            print(f"[kernel] trace: {r.instructions_and_trace[1]}")
    res = r.results
    out = np.concatenate([res[c]["out"] for c in range(n_cores)], axis=0)
    return out.reshape(N, 1, TT, 128)


if __name__ == "__main__":
    a = np.random.randn(16, 32000).astype(np.float32)
    o = kernel(a)
    print("kernel ok", o.shape, o.dtype, float(o.min()), float(o.max()))

